# revision 1
# baseline (speedup 1.0000x reference)
"""Trainium2 Bass kernel: image -> additive-sinusoid audio encoding.

Math (per batch image b):
  gray = 255 * (w . rgb);  rev = flip(gray, rows);  avg = mean(gray)
  px   = clip(3*rev - 2*avg, 0, 255)
  A    = where(px==0, 0, exp(ln10 * (px/160 - 1.5)))            # [M=64 rows, N=64 cols]
  y[t] = sum_m A[m, col(t)] * sin(W[m]*t*dt + PHI0[m]),  col(t) = min(t//361, 63)
  audio= clip(0.5 + 2048*y, -32768, 32767)                       # [ns=23152]

Kernel strategy: t = n*361 + r  =>  angle = theta[i,n] + beta[i,r] (row flip folded
into the host tables), so  sinmat = sin(theta)cos(beta) + cos(theta)sin(beta) and
the gathered einsum becomes dense fp16 matmuls of P/Q = A*sin(theta)/A*cos(theta)
against tiny constant cos/sin(beta) banks. Data-parallel over batch: 8 images per
NeuronCore, layout [128 partitions = (batch-half, image-row), 256 = (b2, col)].
"""

import os

import numpy as np

# ---- problem constants (from the nn.Module definition; input-independent) ----
M = 64
N = 64
FL, FH, FS, T = 80.0, 7600.0, 22050, 1.05
NS = 2 * int(0.5 * FS * T)  # 23152
NUM = NS // N  # 361
RMAX = NS - (N - 1) * NUM  # 409 (last column's sample count)
DT = float(np.float32(1.0 / FS))  # reference rounds dt to f32 (jnp weak typing)
TWO_PI = 2.0 * np.pi
B = 64
N_CORES = 8
B_LOC = B // N_CORES  # 8 images per core
SCALE_SSM = (0.5 / np.sqrt(M)) * 32768.0  # 2048
LN10 = float(np.log(10.0))
EXP_A = LN10 / 160.0
EXP_B = -1.5 * LN10
W0, W1, W2 = 0.2989, 0.5870, 0.1140
C00 = 3.0 * 255.0 * W0  # fold of the 3*255*w0 scale into the gray accumulator
R1 = W1 / W0
R2 = W2 / W0
KAVG2 = 2.0 * 255.0 * W0 / 4096.0  # sum(t) -> 2*avg(gray255) weighting
A255 = float(np.exp(np.float64(EXP_A) * 255.0 + EXP_B))  # A value at px=255


def _make_tables():
    # LCG phase bank (faithful port, ir starts at 0)
    ia, ic, im = 9301, 49297, 233280
    ir = 0
    phi = []
    for _ in range(M):
        ir = (ir * ia + ic) % im
        phi.append(TWO_PI * ir / im)
    phi32 = np.array(phi, np.float64).astype(np.float32)
    w32 = (TWO_PI * FL * (FH / FL) ** (np.arange(M) / (M - 1))).astype(np.float32)

    # fold the row flip (tf.reverse on axis 1) into the tables: row i uses W[63-i]
    wf = w32[::-1].astype(np.float64)
    phif = phi32[::-1].astype(np.float64)

    n_idx = np.arange(N, dtype=np.float64)
    theta = wf[:, None] * (n_idx[None, :] * NUM * DT) + phif[:, None]  # [64, 64]
    st = np.sin(theta)
    ct = np.cos(theta)

    r_idx = np.arange(RMAX, dtype=np.float64)
    beta = wf[:, None] * (r_idx[None, :] * DT)  # [64, 409]
    cb = np.tile((SCALE_SSM * np.cos(beta)).astype(np.float16), (2, 1))  # [128, RMAX]
    sb = np.tile((SCALE_SSM * np.sin(beta)).astype(np.float16), (2, 1))

    # [p=(bh,i), (b2,n)] broadcast of the theta tables
    stbc = np.tile(st[None, :, None, :], (2, 1, 4, 1)).reshape(128, 256)
    ctbc = np.tile(ct[None, :, None, :], (2, 1, 4, 1)).reshape(128, 256)

    # pack1: [stbc(256) | ctbc(256) | expb(1)] fp32
    pack1 = np.zeros((128, 513), np.float32)
    pack1[:, 0:256] = stbc
    pack1[:, 256:512] = ctbc
    pack1[:, 512] = EXP_B

    # pack2 (fp16): [cb | sb | ones_row+half_row(537, partition 0) | bcast128(128)]
    # bcast128[p, m] = KAVG2 * (p//64 == m//64): one matmul does the
    # cross-partition mean reduce AND broadcasts it to all 128 partitions.
    extra = np.zeros((128, 128 + RMAX), np.float16)
    extra[0, 0:128] = 1.0
    extra[0, 128 : 128 + RMAX] = 0.5
    blk = np.zeros((128, 128), np.float16)
    blk[:64, :64] = 1.0
    blk[64:, 64:] = 1.0
    pack2 = np.concatenate([cb, sb, extra, blk.astype(np.float16)], axis=1)

    return {"pack1": pack1, "pack2": pack2}


_TABLES = None


def tables():
    global _TABLES
    if _TABLES is None:
        _TABLES = _make_tables()
    return _TABLES


def build_nc():
    import concourse.bacc as bacc
    import concourse.bass as bass
    import concourse.mybir as mybir
    import concourse.tile as tile

    f32 = mybir.dt.float32
    f16 = mybir.dt.float16
    Alu = mybir.AluOpType
    Act = mybir.ActivationFunctionType

    nc = bacc.Bacc(
        "TRN2",
        target_bir_lowering=False,
        debug=False,
        num_devices=N_CORES,
        enable_asserts=False,
    )

    x_d = nc.dram_tensor("x", [B_LOC, 64, 64, 3], f32, kind="ExternalInput")
    pack1_d = nc.dram_tensor("pack1", [128, 513], f32, kind="ExternalInput")
    pack2_d = nc.dram_tensor(
        "pack2", [128, 3 * RMAX + 256], f16, kind="ExternalInput"
    )
    audio_d = nc.dram_tensor("audio", [B_LOC, NS], f32, kind="ExternalOutput")
    atail_d = nc.dram_tensor(
        "audio_tail", [2, 4, RMAX - NUM], f32, kind="ExternalOutput"
    )

    with tile.TileContext(nc) as tc:
        with (
            tc.tile_pool(name="consts", bufs=1) as consts,
            tc.tile_pool(name="work", bufs=1) as work,
            tc.tile_pool(name="outp", bufs=8) as outp,
            tc.tile_pool(name="psum_y", bufs=4, space=bass.MemorySpace.PSUM) as psum_y,
            tc.tile_pool(name="psum_m", bufs=1, space=bass.MemorySpace.PSUM) as psum_m,
        ):
            # ---- input image: [p=(bh,i), (b2, j, c)]; one DMA per batch half,
            # split across the two physical HWDGE rings (sync / scalar) ----
            X = work.tile([128, 768], f32)
            xv = x_d[:].rearrange("(bh b2) i j c -> bh i b2 j c", bh=2)
            Xv = X[:].rearrange("(bh i) (b2 j c) -> bh i b2 j c", bh=2, b2=4, c=3)
            nc.sync.dma_start(out=Xv[0], in_=xv[0])
            nc.scalar.dma_start(out=Xv[1], in_=xv[1])

            # ---- constant tables (pack2 on the scalar ring, pack1 on SWDGE) ----
            pk1 = consts.tile([128, 513], f32)
            pk2 = consts.tile([128, 3 * RMAX + 256], f16)
            nc.scalar.dma_start(out=pk2, in_=pack2_d[:])
            nc.gpsimd.dma_start(out=pk1, in_=pack1_d[:])
            stbc = pk1[:, 0:256]
            ctbc = pk1[:, 256:512]
            expb = pk1[:, 512:513]
            cb = pk2[:, 0:RMAX]
            sbt = pk2[:, RMAX : 2 * RMAX]
            ones_row = pk2[0:1, 2 * RMAX : 2 * RMAX + 128]
            half_row = pk2[0:1, 2 * RMAX + 128 : 3 * RMAX + 128]
            bcast128 = pk2[:, 3 * RMAX + 128 : 3 * RMAX + 256]

            # ---- PSUM pre-fill: y = 0.5 via K=1 matmul per group; runs during
            # the input/elementwise phase (only depends on pack2) ----
            y_tiles = []
            for g in range(4):
                y_ps = psum_y.tile([128, NUM], f32, tag="y")
                nc.tensor.matmul(y_ps, ones_row, half_row[0:1, 0:NUM], start=True, stop=False)
                y_tiles.append(y_ps)

            # ---- grayscale accumulate: t = R + r1*G + r2*B; rs = per-b2 sums ----
            Xc = X[:].rearrange("p (q c) -> p q c", c=3)
            t = work.tile([128, 4, 64], f32)
            rs = work.tile([128, 4], f32)
            rs16 = work.tile([128, 4], f16)
            nc.vector.scalar_tensor_tensor(
                out=t.rearrange("p a b -> p (a b)"),
                in0=Xc[:, :, 1], scalar=float(R1), in1=Xc[:, :, 0],
                op0=Alu.mult, op1=Alu.add,
            )
            nc.vector.scalar_tensor_tensor(
                out=t.rearrange("p a b -> p (a b)"), in0=Xc[:, :, 2],
                scalar=float(R2), in1=t.rearrange("p a b -> p (a b)"),
                op0=Alu.mult, op1=Alu.add,
            )
            nc.vector.reduce_sum(out=rs, in_=t, axis=mybir.AxisListType.X)
            nc.vector.tensor_scalar_mul(out=rs16, in0=rs, scalar1=float(KAVG2))

            # ---- mean: one matmul reduces across partitions AND broadcasts:
            # csS2[p, b2] = KAVG2 * sum_{p' in half(p)} rs[p', b2] = 2*avg ----
            csS2 = psum_m.tile([128, 4], f32)
            nc.tensor.matmul(csS2, bcast128, rs16, start=True, stop=True)

            # ---- per column-half s (b2 pair): px -> A -> P/Q -> matmul -> out
            # (pipelines the s=1 elementwise under s=0's PE/DMA work) ----
            px = work.tile([128, 4, 64], f32)
            E = work.tile([128, 4, 64], f32)
            A = work.tile([128, 4, 64], f32)
            P = work.tile([128, 256], f16)
            Q = work.tile([128, 256], f16)
            tailps = psum_m.tile([2, 4, RMAX - NUM], f32)
            Pv = P[:].rearrange("p (a b) -> p a b", b=64)
            Qv = Q[:].rearrange("p (a b) -> p a b", b=64)
            for s in range(2):
                b2s = slice(2 * s, 2 * s + 2)
                nc.vector.scalar_tensor_tensor(
                    out=px[:, b2s], in0=t[:, b2s], scalar=float(C00),
                    in1=csS2[:, b2s].broadcast_to([128, 2, 64]),
                    op0=Alu.mult, op1=Alu.subtract,
                )
                nc.vector.tensor_scalar(
                    out=px[:, b2s], in0=px[:, b2s], scalar1=0.0, scalar2=255.0,
                    op0=Alu.max, op1=Alu.min,
                )
                nc.scalar.activation(
                    out=E[:, b2s], in_=px[:, b2s], func=Act.Exp,
                    bias=expb, scale=float(EXP_A),
                )
                nc.vector.scalar_tensor_tensor(
                    out=A[:, b2s], in0=px[:, b2s], scalar=0.0, in1=E[:, b2s],
                    op0=Alu.is_gt, op1=Alu.mult,
                )
                nc.vector.tensor_mul(
                    out=Pv[:, b2s], in0=A[:, b2s],
                    in1=stbc.rearrange("p (a b) -> p a b", b=64)[:, b2s],
                )
                nc.gpsimd.tensor_mul(
                    out=Qv[:, b2s], in0=A[:, b2s],
                    in1=ctbc.rearrange("p (a b) -> p a b", b=64)[:, b2s],
                )

                col = slice(128 * s, 128 * (s + 1))
                for bh in range(2):
                    g = 2 * bh + s
                    prt = slice(64 * bh, 64 * (bh + 1))
                    y_ps = y_tiles[g]
                    nc.tensor.matmul(
                        y_ps, P[prt, col], cb[prt, 0:NUM],
                        start=False, stop=False,
                    )
                    nc.tensor.matmul(
                        y_ps, Q[prt, col], sbt[prt, 0:NUM],
                        start=False, stop=True,
                    )

                    # tail samples (n=63, r>=361): tiny matmuls on the
                    # n=63 columns of P/Q into a dedicated [2, g, 48] psum
                    ctail = slice(128 * s + 63, 128 * s + 128, 64)
                    nc.tensor.matmul(
                        tailps[:, g], P[prt, ctail], cb[prt, NUM:RMAX],
                        start=True, stop=False,
                    )
                    nc.tensor.matmul(
                        tailps[:, g], Q[prt, ctail], sbt[prt, NUM:RMAX],
                        start=False, stop=True,
                    )

                    u = outp.tile([128, NUM], f32, tag="u")
                    nc.vector.tensor_scalar(
                        out=u, in0=y_ps[:, 0:NUM],
                        scalar1=-32768.0, scalar2=32767.0,
                        op0=Alu.max, op1=Alu.min,
                    )
                    b0 = 4 * bh + 2 * s
                    # main blocks: one DMA per batch-half; keep the 64-wide n
                    # dim outermost (HWDGE fans descriptors over the 16 SDMA
                    # engines by the DRAM-side outer dim)
                    for half, eng in ((0, nc.sync), (1, nc.scalar)):
                        eng.dma_start(
                            out=bass.AP(
                                audio_d, (b0 + half) * NS, [[NUM, 64], [1, NUM]]
                            ),
                            in_=u[64 * half : 64 * (half + 1), :],
                        )
            # clip + store all 8 tails (batch b = 2g+l) in one op + one DMA
            TTs = outp.tile([2, 4, RMAX - NUM], f32)
            nc.vector.tensor_scalar(
                out=TTs, in0=tailps, scalar1=0.5, scalar2=-32768.0,
                op0=Alu.add, op1=Alu.max,
            )
            nc.vector.tensor_scalar_min(out=TTs, in0=TTs, scalar1=32767.0)
            nc.sync.dma_start(out=atail_d[:], in_=TTs)

    nc.compile()
    return nc


_NC = None


def _get_nc():
    global _NC
    if _NC is None:
        _NC = build_nc()
    return _NC


LAST_RESULTS = None


def kernel(x: np.ndarray) -> np.ndarray:
    from concourse.bass_utils import run_bass_kernel_spmd

    x = np.ascontiguousarray(np.asarray(x, dtype=np.float32))
    assert x.shape == (B, 64, 64, 3), x.shape

    nc = _get_nc()
    tbl = tables()
    in_maps = []
    for c in range(N_CORES):
        m = {"x": np.ascontiguousarray(x[c * B_LOC : (c + 1) * B_LOC])}
        m.update(tbl)
        in_maps.append(m)

    trace = os.environ.get("BASS_KERNEL_TRACE", "0") == "1"
    res = run_bass_kernel_spmd(
        nc, in_maps, core_ids=list(range(N_CORES)), trace=trace
    )
    global LAST_RESULTS
    LAST_RESULTS = res
    outs = []
    for r in res.results:
        a = r["audio"].copy()
        tt = r["audio_tail"]  # [l, g, 48]; batch b = 2*g + l
        a[:, N * NUM :] = tt.transpose(1, 0, 2).reshape(B_LOC, RMAX - NUM)
        outs.append(a)
    return np.concatenate(outs, axis=0)



# revision 14
# speedup vs baseline: 1.1453x; 1.1453x over previous
"""Trainium2 Bass kernel: image -> additive-sinusoid audio encoding.

Math (per batch image b):
  gray = 255 * (w . rgb);  rev = flip(gray, rows);  avg = mean(gray)
  px   = clip(3*rev - 2*avg, 0, 255)
  A    = where(px==0, 0, exp(ln10 * (px/160 - 1.5)))            # [M=64 rows, N=64 cols]
  y[t] = sum_m A[m, col(t)] * sin(W[m]*t*dt + PHI0[m]),  col(t) = min(t//361, 63)
  audio= clip(0.5 + 2048*y, -32768, 32767)                       # [ns=23152]

Kernel strategy: t = n*361 + r  =>  angle = theta[i,n] + beta[i,r] (row flip folded
into the host tables), so  sinmat = sin(theta)cos(beta) + cos(theta)sin(beta).
P = A*sin(theta), Q = A*cos(theta) are fused into ONE K=128 matmul per image pair:
A is duplicated onto both partition halves with a tiny permutation matmul
(PE-dup; each matmul output gets its own psum tile - outputs at a free-offset
inside a tile fault on HW), then per pair
  y[(b2',n), r] = sum_{k<64} PQ[k]*cb[k,r] + sum_{k>=64} PQ[k]*sb[k-64,r]
with r running to 409 so the col-63 tail samples fall out of the same matmul.
Data-parallel over batch: 8 images per NeuronCore, layout
[128 partitions = (batch-half bh, image-row i), free = (b2, col)].
Host side: input is pre-permuted to the exact SBUF layout in fp16; output comes
back as fp16 y+0.5 and is clipped/cast during the unshard gather.
"""

import os

import numpy as np

# ---- problem constants (from the nn.Module definition; input-independent) ----
M = 64
N = 64
FL, FH, FS, T = 80.0, 7600.0, 22050, 1.05
NS = 2 * int(0.5 * FS * T)  # 23152
NUM = NS // N  # 361
RMAX = NS - (N - 1) * NUM  # 409 (last column's sample count)
NTAIL = RMAX - NUM  # 48
DT = float(np.float32(1.0 / FS))  # reference rounds dt to f32 (jnp weak typing)
TWO_PI = 2.0 * np.pi
B = 64
N_CORES = 8
B_LOC = B // N_CORES  # 8 images per core
SCALE_SSM = (0.5 / np.sqrt(M)) * 32768.0  # 2048
LN10 = float(np.log(10.0))
EXP_A = LN10 / 160.0
EXP_B = -1.5 * LN10
W0, W1, W2 = 0.2989, 0.5870, 0.1140
C00 = 3.0 * 255.0 * W0  # fold of the 3*255*w0 scale into the gray accumulator
R1 = W1 / W0
R2 = W2 / W0
KAVG2 = 2.0 * 255.0 * W0 / 4096.0  # sum(t) -> 2*avg(gray255) weighting

# single table: [BC 128 | T 128 | PERM 128 | CS 409 | pad 1]
TABW = 128 + 128 + 128 + RMAX + 1  # pad keeps row stride 4B-aligned for LDWEIGHTS
C_BC, C_T, C_PM, C_CS = 0, 128, 256, 384


def _make_tables():
    # LCG phase bank (faithful port, ir starts at 0)
    ia, ic, im = 9301, 49297, 233280
    ir = 0
    phi = []
    for _ in range(M):
        ir = (ir * ia + ic) % im
        phi.append(TWO_PI * ir / im)
    phi32 = np.array(phi, np.float64).astype(np.float32)
    w32 = (TWO_PI * FL * (FH / FL) ** (np.arange(M) / (M - 1))).astype(np.float32)

    # fold the row flip (tf.reverse on axis 1) into the tables: row i uses W[63-i]
    wf = w32[::-1].astype(np.float64)
    phif = phi32[::-1].astype(np.float64)

    n_idx = np.arange(N, dtype=np.float64)
    theta = wf[:, None] * (n_idx[None, :] * NUM * DT) + phif[:, None]  # [64, 64]
    stct = np.concatenate([np.sin(theta), np.cos(theta)], axis=0)  # [128, 64]
    # T[p, (b2', n)]: sin/cos(theta) broadcast over the 2 images of a pair
    Tt = np.tile(stct[:, None, :], (1, 2, 1)).reshape(128, 128)

    r_idx = np.arange(RMAX, dtype=np.float64)
    beta = wf[:, None] * (r_idx[None, :] * DT)  # [64, 409]
    cs = np.concatenate(
        [SCALE_SSM * np.cos(beta), SCALE_SSM * np.sin(beta)], axis=0
    )  # [128, 409]

    pp = np.arange(128)
    perm = (pp[:, None] % 64 == pp[None, :] % 64).astype(np.float64)  # [128,128]
    bc = KAVG2 * (pp[:, None] // 64 == pp[None, :] // 64).astype(np.float64)

    pad = np.zeros((128, 1))
    tabs = np.concatenate([bc, Tt, perm, cs, pad], axis=1).astype(np.float16)
    assert tabs.shape == (128, TABW), tabs.shape
    return {"tabs": np.ascontiguousarray(tabs)}


_TABLES = None


def tables():
    global _TABLES
    if _TABLES is None:
        _TABLES = _make_tables()
    return _TABLES


def build_nc():
    import concourse.bacc as bacc
    import concourse.bass as bass
    import concourse.mybir as mybir
    import concourse.tile as tile

    f32 = mybir.dt.float32
    f16 = mybir.dt.float16
    Alu = mybir.AluOpType
    Act = mybir.ActivationFunctionType

    nc = bacc.Bacc(
        "TRN2",
        target_bir_lowering=False,
        debug=False,
        num_devices=N_CORES,
        enable_asserts=False,
    )

    x16_d = nc.dram_tensor("x16", [128, 768], f16, kind="ExternalInput")
    tabs_d = nc.dram_tensor("tabs", [128, TABW], f16, kind="ExternalInput")
    audio_d = nc.dram_tensor("audio16", [4, 128, RMAX], f16, kind="ExternalOutput")

    with tile.TileContext(nc) as tc:
        with (
            tc.tile_pool(name="work", bufs=1) as work,
            tc.tile_pool(name="psum", bufs=1, space=bass.MemorySpace.PSUM) as psum,
        ):
            # Exp activation bias column (scalar engine requires an AP bias)
            expb = work.tile([128, 1], f32)
            nc.gpsimd.memset(expb, float(EXP_B))

            # ---- input DMAs: X pre-permuted on host to the exact SBUF layout
            # [p=(bh,i), (c, b2, j)]; halves on the two HWDGE rings, tables
            # in one SWDGE transfer ----
            X = work.tile([128, 768], f16)
            TB = work.tile([128, TABW], f16)
            nc.sync.dma_start(out=X[0:64], in_=x16_d[0:64])
            nc.scalar.dma_start(out=X[64:128], in_=x16_d[64:128])
            nc.gpsimd.dma_start(out=TB, in_=tabs_d[:])
            BC = TB[:, C_BC : C_BC + 128]
            Tt = TB[:, C_T : C_T + 128]
            CS = TB[:, C_CS : C_CS + RMAX]

            # ---- grayscale accumulate on DVE: t = R + r1*G + r2*B ----
            Xv = X[:].rearrange("p (c q) -> p c q", c=3)
            t = work.tile([128, 256], f32)
            nc.vector.scalar_tensor_tensor(
                out=t, in0=Xv[:, 1], scalar=float(R1), in1=Xv[:, 0],
                op0=Alu.mult, op1=Alu.add,
            )
            nc.vector.scalar_tensor_tensor(
                out=t, in0=Xv[:, 2], scalar=float(R2), in1=t,
                op0=Alu.mult, op1=Alu.add,
            )

            # ---- mean path: per-row sums on DVE, f16 cast on GpSimd ----
            rs = work.tile([128, 4], f32)
            rs16 = work.tile([128, 4], f16)
            nc.vector.reduce_sum(
                out=rs, in_=t[:].rearrange("p (b j) -> p b j", j=64),
                axis=mybir.AxisListType.X,
            )
            nc.gpsimd.tensor_scalar_mul(out=rs16, in0=rs, scalar1=1.0)
            # one matmul: cross-partition per-half sum AND broadcast to all 128
            csS2 = psum.tile([128, 4], f32)
            nc.tensor.matmul(csS2, BC, rs16, start=True, stop=True)

            # ---- per image-pair s: px -> E -> A ----
            px = [work.tile([128, 2, 64], f32, name=f"px{s}") for s in range(2)]
            E = [work.tile([128, 128], f16, name=f"E{s}") for s in range(2)]
            A16 = [work.tile([128, 128], f16, name=f"A16_{s}") for s in range(2)]
            tv = t[:].rearrange("p (b q) -> p b q", b=4)

            for s in range(2):
                nc.vector.scalar_tensor_tensor(
                    out=px[s], in0=tv[:, 2 * s : 2 * s + 2], scalar=float(C00),
                    in1=csS2[:, 2 * s : 2 * s + 2].broadcast_to([128, 2, 64]),
                    op0=Alu.mult, op1=Alu.subtract,
                )
                nc.vector.tensor_scalar(
                    out=px[s], in0=px[s], scalar1=0.0, scalar2=255.0,
                    op0=Alu.max, op1=Alu.min,
                )
                nc.scalar.activation(
                    out=E[s], in_=px[s].rearrange("p a b -> p (a b)"),
                    func=Act.Exp, bias=expb, scale=float(EXP_A),
                )

            # ---- A masks + PE duplication (all four halves up front) ----
            a2 = {}
            for s in range(2):
                nc.vector.scalar_tensor_tensor(
                    out=A16[s], in0=px[s].rearrange("p a b -> p (a b)"),
                    scalar=0.0, in1=E[s], op0=Alu.is_gt, op1=Alu.mult,
                )
                a2[s, 0] = psum.tile([128, 128], f32, name=f"a2lo{s}")
                a2[s, 1] = psum.tile([128, 128], f32, name=f"a2hi{s}")
                nc.tensor.matmul(
                    a2[s, 0], TB[0:64, C_PM : C_PM + 128], A16[s][0:64],
                    start=True, stop=True,
                )
                nc.tensor.matmul(
                    a2[s, 1], TB[64:128, C_PM : C_PM + 128], A16[s][64:128],
                    start=True, stop=True,
                )

            # ---- PQ = A2 * T, then one K=128 N=409 matmul per image pair ----
            PQ = [
                work.tile([128, 128], f16, name=f"PQ{s}{h}")
                for s in range(2) for h in range(2)
            ]
            y = {}
            for g in (0, 2, 1, 3):  # (bh, s) completion order
                bh, s = g // 2, g % 2
                pq = PQ[2 * s + bh]
                nc.vector.tensor_mul(out=pq, in0=a2[s, bh], in1=Tt)
                yt = psum.tile([128, RMAX], f32, tag=f"y{g // 2}", bufs=1, name=f"y{g}")
                nc.tensor.matmul(yt, pq, CS, start=True, stop=True)
                y[g] = yt

            # ---- PSUM drain: u = y + 0.5, fp16; clip happens on host where
            # clip(0.5+v,lo,hi) == 0.5+clip(v,lo-.5,hi-.5) (f16 inf clips too) ----
            u = [work.tile([128, RMAX], f16, name=f"u{g}") for g in range(4)]
            for g, eng in ((0, "s"), (2, "v"), (1, "s"), (3, "v")):
                if eng == "s":
                    nc.scalar.activation(
                        out=u[g], in_=y[g], func=Act.Copy, bias=0.5, scale=1.0,
                    )
                else:
                    nc.vector.tensor_scalar(
                        out=u[g], in0=y[g], scalar1=0.5, scalar2=0.0,
                        op0=Alu.add, op1=Alu.bypass,
                    )
            for g, eng in ((0, nc.sync), (2, nc.scalar), (1, nc.sync), (3, nc.scalar)):
                eng.dma_start(out=audio_d[g], in_=u[g])

    nc.compile()
    return nc


_NC = None


def _get_nc():
    global _NC
    if _NC is None:
        _NC = build_nc()
    return _NC


LAST_RESULTS = None


def kernel(x: np.ndarray) -> np.ndarray:
    from concourse.bass_utils import run_bass_kernel_spmd

    x = np.asarray(x, dtype=np.float32)
    assert x.shape == (B, 64, 64, 3), x.shape

    # shard + permute to the SBUF layout [p=(bh,i), (c, b2, j)], fp16
    xc = x.reshape(N_CORES, 2, 4, 64, 64, 3)  # [core, bh, b2, i, j, c]
    x16 = xc.transpose(0, 1, 3, 5, 2, 4).reshape(N_CORES, 128, 768)
    x16 = np.ascontiguousarray(x16).astype(np.float16)

    nc = _get_nc()
    tbl = tables()
    in_maps = []
    for c in range(N_CORES):
        m = {"x16": x16[c]}
        m.update(tbl)
        in_maps.append(m)

    trace = os.environ.get("BASS_KERNEL_TRACE", "0") == "1"
    res = run_bass_kernel_spmd(
        nc, in_maps, core_ids=list(range(N_CORES)), trace=trace
    )
    global LAST_RESULTS
    LAST_RESULTS = res

    outs = np.empty((B, NS), np.float32)
    for c, r in enumerate(res.results):
        # audio16[g=(bh,s), p=(b2',n), r<=409]; b_loc = 4bh + 2s + b2'
        a = r["audio16"].astype(np.float32).reshape(2, 2, 2, 64, RMAX)
        outs[c * B_LOC : (c + 1) * B_LOC, : N * NUM] = a[..., :NUM].reshape(
            B_LOC, N * NUM
        )
        # col-63 tail: samples 23104..23151 use the r>=361 range of column 63
        outs[c * B_LOC : (c + 1) * B_LOC, N * NUM :] = a[:, :, :, 63, NUM:].reshape(
            B_LOC, NTAIL
        )
    np.clip(outs, -32768.0, 32767.0, out=outs)
    return outs


# revision 15
# speedup vs baseline: 1.1847x; 1.0344x over previous
"""Trainium2 Bass kernel: image -> additive-sinusoid audio encoding.

Math (per batch image b):
  gray = 255 * (w . rgb);  rev = flip(gray, rows);  avg = mean(gray)
  px   = clip(3*rev - 2*avg, 0, 255)
  A    = where(px==0, 0, exp(ln10 * (px/160 - 1.5)))            # [M=64 rows, N=64 cols]
  y[t] = sum_m A[m, col(t)] * sin(W[m]*t*dt + PHI0[m]),  col(t) = min(t//361, 63)
  audio= clip(0.5 + 2048*y, -32768, 32767)                       # [ns=23152]

Kernel strategy: t = n*361 + r  =>  angle = theta[i,n] + beta[i,r] (row flip folded
into the host tables), so  sinmat = sin(theta)cos(beta) + cos(theta)sin(beta).
P = A*sin(theta), Q = A*cos(theta) are fused into ONE K=128 matmul per image pair:
A is duplicated onto both partition halves with a tiny permutation matmul
(PE-dup; each matmul output gets its own psum tile - outputs at a free-offset
inside a tile fault on HW), then per pair
  y[(b2',n), r] = sum_{k<64} PQ[k]*cb[k,r] + sum_{k>=64} PQ[k]*sb[k-64,r]
with r running to 409 so the col-63 tail samples fall out of the same matmul.
Data-parallel over batch: 8 images per NeuronCore, layout
[128 partitions = (batch-half bh, image-row i), free = (b2, col)].
Host side: input is pre-permuted to the exact SBUF layout in fp16; output comes
back as fp16 y+0.5 and is clipped/cast during the unshard gather.
"""

import os

import numpy as np

# ---- problem constants (from the nn.Module definition; input-independent) ----
M = 64
N = 64
FL, FH, FS, T = 80.0, 7600.0, 22050, 1.05
NS = 2 * int(0.5 * FS * T)  # 23152
NUM = NS // N  # 361
RMAX = NS - (N - 1) * NUM  # 409 (last column's sample count)
NTAIL = RMAX - NUM  # 48
DT = float(np.float32(1.0 / FS))  # reference rounds dt to f32 (jnp weak typing)
TWO_PI = 2.0 * np.pi
B = 64
N_CORES = 8
B_LOC = B // N_CORES  # 8 images per core
SCALE_SSM = (0.5 / np.sqrt(M)) * 32768.0  # 2048
LN10 = float(np.log(10.0))
EXP_A = LN10 / 160.0
EXP_B = -1.5 * LN10
W0, W1, W2 = 0.2989, 0.5870, 0.1140
C00 = 3.0 * 255.0 * W0  # fold of the 3*255*w0 scale into the gray accumulator
R1 = W1 / W0
R2 = W2 / W0
KAVG2 = 2.0 * 255.0 * W0 / 4096.0  # sum(t) -> 2*avg(gray255) weighting

# single table: [BCW 3*128 | T 128 | PERM 128 | CS 409 | pad 1]
TABW = 384 + 128 + 128 + RMAX + 1  # pad keeps row stride 4B-aligned for LDWEIGHTS
C_BC, C_T, C_PM, C_CS = 0, 384, 512, 640


def _make_tables():
    # LCG phase bank (faithful port, ir starts at 0)
    ia, ic, im = 9301, 49297, 233280
    ir = 0
    phi = []
    for _ in range(M):
        ir = (ir * ia + ic) % im
        phi.append(TWO_PI * ir / im)
    phi32 = np.array(phi, np.float64).astype(np.float32)
    w32 = (TWO_PI * FL * (FH / FL) ** (np.arange(M) / (M - 1))).astype(np.float32)

    # fold the row flip (tf.reverse on axis 1) into the tables: row i uses W[63-i]
    wf = w32[::-1].astype(np.float64)
    phif = phi32[::-1].astype(np.float64)

    n_idx = np.arange(N, dtype=np.float64)
    theta = wf[:, None] * (n_idx[None, :] * NUM * DT) + phif[:, None]  # [64, 64]
    stct = np.concatenate([np.sin(theta), np.cos(theta)], axis=0)  # [128, 64]
    # T[p, (b2', n)]: sin/cos(theta) broadcast over the 2 images of a pair
    Tt = np.tile(stct[:, None, :], (1, 2, 1)).reshape(128, 128)

    r_idx = np.arange(RMAX, dtype=np.float64)
    beta = wf[:, None] * (r_idx[None, :] * DT)  # [64, 409]
    cs = np.concatenate(
        [SCALE_SSM * np.cos(beta), SCALE_SSM * np.sin(beta)], axis=0
    )  # [128, 409]

    pp = np.arange(128)
    perm = (pp[:, None] % 64 == pp[None, :] % 64).astype(np.float64)  # [128,128]
    blk = (pp[:, None] // 64 == pp[None, :] // 64).astype(np.float64)
    # three c-weighted mean blocks: csS2 = sum_c (KAVG2*w'_c*blk)^T . rsx_c
    bcw = np.concatenate(
        [KAVG2 * 1.0 * blk, KAVG2 * R1 * blk, KAVG2 * R2 * blk], axis=1
    )

    pad = np.zeros((128, 1))
    tabs = np.concatenate([bcw, Tt, perm, cs, pad], axis=1).astype(np.float16)
    assert tabs.shape == (128, TABW), tabs.shape
    return {"tabs": np.ascontiguousarray(tabs)}


_TABLES = None


def tables():
    global _TABLES
    if _TABLES is None:
        _TABLES = _make_tables()
    return _TABLES


def build_nc():
    import concourse.bacc as bacc
    import concourse.bass as bass
    import concourse.mybir as mybir
    import concourse.tile as tile

    f32 = mybir.dt.float32
    f16 = mybir.dt.float16
    Alu = mybir.AluOpType
    Act = mybir.ActivationFunctionType

    nc = bacc.Bacc(
        "TRN2",
        target_bir_lowering=False,
        debug=False,
        num_devices=N_CORES,
        enable_asserts=False,
    )

    x16_d = nc.dram_tensor("x16", [128, 768], f16, kind="ExternalInput")
    tabs_d = nc.dram_tensor("tabs", [128, TABW], f16, kind="ExternalInput")
    audio_a_d = nc.dram_tensor("audio_a", [128, 2, RMAX], f16, kind="ExternalOutput")
    audio_b_d = nc.dram_tensor("audio_b", [128, 2, RMAX], f16, kind="ExternalOutput")

    with tile.TileContext(nc) as tc:
        with (
            tc.tile_pool(name="work", bufs=1) as work,
            tc.tile_pool(name="psum", bufs=1, space=bass.MemorySpace.PSUM) as psum,
        ):
            # Exp activation bias column (scalar engine requires an AP bias)
            expb = work.tile([128, 1], f32)
            nc.vector.memset(expb, float(EXP_B))

            # ---- input DMAs: X pre-permuted on host to the exact SBUF layout
            # [p=(bh,i), (c, b2, j)]; halves on the two HWDGE rings, tables
            # in one SWDGE transfer ----
            X = work.tile([128, 768], f16)
            TB = work.tile([128, TABW], f16)
            nc.sync.dma_start(out=X[0:64], in_=x16_d[0:64])
            nc.scalar.dma_start(out=X[64:128], in_=x16_d[64:128])
            nc.gpsimd.dma_start(out=TB, in_=tabs_d[:])
            Tt = TB[:, C_T : C_T + 128]
            CS = TB[:, C_CS : C_CS + RMAX]

            # ---- mean path first: channel row-sums on DVE, then 3 weighted
            # block matmuls reduce across partitions AND broadcast, all while
            # DVE moves on to the gray accumulate ----
            rsx = work.tile([128, 12], f16)
            with nc.allow_low_precision("mean row-sums <=64 fit f16"):
                nc.vector.reduce_sum(
                    out=rsx, in_=X[:].rearrange("p (cb j) -> p cb j", j=64),
                    axis=mybir.AxisListType.X,
                )
            csS2 = psum.tile([128, 4], f32)
            for c in range(3):
                nc.tensor.matmul(
                    csS2, TB[:, C_BC + 128 * c : C_BC + 128 * (c + 1)],
                    rsx[:, 4 * c : 4 * (c + 1)],
                    start=(c == 0), stop=(c == 2),
                )

            # ---- grayscale accumulate on DVE: t = R + r1*G + r2*B ----
            Xv = X[:].rearrange("p (c q) -> p c q", c=3)
            t = work.tile([128, 256], f32)
            nc.vector.scalar_tensor_tensor(
                out=t, in0=Xv[:, 1], scalar=float(R1), in1=Xv[:, 0],
                op0=Alu.mult, op1=Alu.add,
            )
            nc.vector.scalar_tensor_tensor(
                out=t, in0=Xv[:, 2], scalar=float(R2), in1=t,
                op0=Alu.mult, op1=Alu.add,
            )

            # ---- per image-pair s: px -> E -> A ----
            px = [work.tile([128, 2, 64], f32, name=f"px{s}") for s in range(2)]
            E = [work.tile([128, 128], f16, name=f"E{s}") for s in range(2)]
            A16 = [work.tile([128, 128], f16, name=f"A16_{s}") for s in range(2)]
            tv = t[:].rearrange("p (b q) -> p b q", b=4)

            for s in range(2):
                nc.vector.scalar_tensor_tensor(
                    out=px[s], in0=tv[:, 2 * s : 2 * s + 2], scalar=float(C00),
                    in1=csS2[:, 2 * s : 2 * s + 2].broadcast_to([128, 2, 64]),
                    op0=Alu.mult, op1=Alu.subtract,
                )
                nc.vector.tensor_scalar(
                    out=px[s], in0=px[s], scalar1=0.0, scalar2=255.0,
                    op0=Alu.max, op1=Alu.min,
                )
                nc.scalar.activation(
                    out=E[s], in_=px[s].rearrange("p a b -> p (a b)"),
                    func=Act.Exp, bias=expb, scale=float(EXP_A),
                )

            # ---- A masks + PE duplication (all four halves up front) ----
            a2 = {}
            for s in range(2):
                nc.vector.scalar_tensor_tensor(
                    out=A16[s], in0=px[s].rearrange("p a b -> p (a b)"),
                    scalar=0.0, in1=E[s], op0=Alu.is_gt, op1=Alu.mult,
                )
                a2[s, 0] = psum.tile([128, 128], f32, name=f"a2lo{s}")
                a2[s, 1] = psum.tile([128, 128], f32, name=f"a2hi{s}")
                nc.tensor.matmul(
                    a2[s, 0], TB[0:64, C_PM : C_PM + 128], A16[s][0:64],
                    start=True, stop=True,
                )
                nc.tensor.matmul(
                    a2[s, 1], TB[64:128, C_PM : C_PM + 128], A16[s][64:128],
                    start=True, stop=True,
                )

            # ---- PQ = A2 * T, then one K=128 N=409 matmul per image pair ----
            PQ = [
                work.tile([128, 128], f16, name=f"PQ{s}{h}")
                for s in range(2) for h in range(2)
            ]
            y = {}
            for g in (0, 2, 1, 3):  # (bh, s) completion order
                bh, s = g // 2, g % 2
                pq = PQ[2 * s + bh]
                nc.vector.tensor_mul(out=pq, in0=a2[s, bh], in1=Tt)
                ytag = {0: "ya", 2: "yb", 1: "yc", 3: "ya"}[g]
                yt = psum.tile([128, RMAX], f32, tag=ytag, bufs=1, name=f"y{g}")
                nc.tensor.matmul(yt, pq, CS, start=True, stop=True)
                y[g] = yt

            # ---- PSUM drain: u = y + 0.5, fp16; clip happens on host where
            # clip(0.5+v,lo,hi) == 0.5+clip(v,lo-.5,hi-.5) (f16 inf clips too) ----
            Ua = work.tile([128, 2, RMAX], f16)
            Ub = work.tile([128, 2, RMAX], f16)
            slot = {0: (Ua, 0), 2: (Ua, 1), 1: (Ub, 0), 3: (Ub, 1)}
            for g, eng in ((0, "s"), (2, "v"), (1, "s"), (3, "v")):
                U, k = slot[g]
                if eng == "s":
                    nc.scalar.activation(
                        out=U[:, k], in_=y[g], func=Act.Copy, bias=0.5, scale=1.0,
                    )
                else:
                    nc.vector.tensor_scalar(
                        out=U[:, k], in0=y[g], scalar1=0.5, scalar2=0.0,
                        op0=Alu.add, op1=Alu.bypass,
                    )
            nc.sync.dma_start(out=audio_a_d[:], in_=Ua)
            nc.scalar.dma_start(out=audio_b_d[:], in_=Ub)

    nc.compile()
    return nc


_NC = None


def _get_nc():
    global _NC
    if _NC is None:
        _NC = build_nc()
    return _NC


LAST_RESULTS = None


def kernel(x: np.ndarray) -> np.ndarray:
    from concourse.bass_utils import run_bass_kernel_spmd

    x = np.asarray(x, dtype=np.float32)
    assert x.shape == (B, 64, 64, 3), x.shape

    # shard + permute to the SBUF layout [p=(bh,i), (c, b2, j)], fp16
    xc = x.reshape(N_CORES, 2, 4, 64, 64, 3)  # [core, bh, b2, i, j, c]
    x16 = xc.transpose(0, 1, 3, 5, 2, 4).reshape(N_CORES, 128, 768)
    x16 = np.ascontiguousarray(x16).astype(np.float16)

    nc = _get_nc()
    tbl = tables()
    in_maps = []
    for c in range(N_CORES):
        m = {"x16": x16[c]}
        m.update(tbl)
        in_maps.append(m)

    trace = os.environ.get("BASS_KERNEL_TRACE", "0") == "1"
    res = run_bass_kernel_spmd(
        nc, in_maps, core_ids=list(range(N_CORES)), trace=trace
    )
    global LAST_RESULTS
    LAST_RESULTS = res

    outs = np.empty((B, NS), np.float32)
    for c, r in enumerate(res.results):
        # audio_a[p=(b2',n), {g0,g2}, r], audio_b[p, {g1,g3}, r]
        # g = 2bh+s; b_loc = 4bh + 2s + b2'
        a = r["audio_a"].astype(np.float32).reshape(2, 64, 2, RMAX)
        b = r["audio_b"].astype(np.float32).reshape(2, 64, 2, RMAX)
        # [b2', n, k, r] -> per image
        for b_loc, (arr, k) in enumerate(
            [(a, 0), (a, 0), (b, 0), (b, 0), (a, 1), (a, 1), (b, 1), (b, 1)]
        ):
            img = arr[b_loc % 2, :, k]  # [64, RMAX]
            row = c * B_LOC + b_loc
            outs[row, : N * NUM] = img[:, :NUM].reshape(N * NUM)
            outs[row, N * NUM :] = img[63, NUM:]
    np.clip(outs, -32768.0, 32767.0, out=outs)
    return outs
